# revision 5
# baseline (speedup 1.0000x reference)
"""3-layer GAT (PyG GATConv, heads=1) on 8 trn2 NeuronCores.

Sharding strategy (per the spec hint): destination-node sharding with edge
partitioning by destination, replicated small parameters, and halo exchange
of gathered source features per partition. Nodes are sorted by in-degree
(self-loops included) and dealt round-robin to the 8 cores, so per-core
edge counts balance and every 128-node tile has near-uniform degree. Per
tile, in-edges form a dense [128 nodes, D_t] slot grid (D_t = max degree in
the tile). The halo exchange materializes, per core, the gathered source
features for its edge partition in slot-grid order, so each device reads
its edge features with full-bandwidth affine DMA; the device performs all
matmuls, attention softmax (per-partition along the free dim), the
weighted aggregation (log-tree over slots), BN folding and activations.
The layer boundary (exchange of the 12.5k-per-core layer outputs into the
per-partition edge halos) is the host-mediated shard/unshard step between
the three per-layer device launches.

Algebraic rewrites vs the reference (fp-equivalent):
 - alpha_src = h @ a_src = x @ (W a_src): attention scalars are node-level
   matvecs, packed as extra columns into the halo rows for layers 2/3;
   layer 1 computes them with an on-chip dot over the halo rows.
 - Layer-2 aggregation in INPUT space: sum_e a_e (xW)[src] = (sum_e a_e
   x[src]) W -> halo rows are 128 wide instead of 256.
 - softmax max-subtraction dropped (|logits| = O(10); exp safe in fp32);
   the denominator divides the aggregate once per node.
 - eval-mode BN + bias folded into per-feature scale/shift vectors.

Note on the on-device alternative: indexed gathers were implemented and
measured on this stack both via DGE vector-dynamic-offsets (correct but
~0.6 us per 512B row, descriptor-fetch serialized) and via the GPSIMD
dma_gather ucode (device-fatal under this runtime). Neither reaches the
memory roofline, so the halo exchange is done host-side as the hint
suggests.
"""
import sys
sys.path.insert(0, "/opt/trn_rl_repo")
import numpy as np

from concourse import bass, bacc, mybir, tile
from concourse import bass_utils

dt = mybir.dt
P = 128
NCORES = 8
EPS = 1e-5
NEG_SLOPE = 0.2
BIG = 1e30

F_IN = 128
H1 = 128
H2 = 256
C = 40
CP = 64          # layer-3 halo row width (40 feats + asrc + adst + pad)
XW = 132         # layer-2 halo row width (128 feats + asrc + adst + pad)


# ----------------------------------------------------------------- host prep

def _prep(x, edge_index):
    N = x.shape[0]
    e0 = np.asarray(edge_index[0], dtype=np.int64)
    e1 = np.asarray(edge_index[1], dtype=np.int64)
    loop = np.arange(N, dtype=np.int64)
    src = np.concatenate([e0, loop])
    dst = np.concatenate([e1, loop])

    deg = np.bincount(dst, minlength=N).astype(np.int64)
    order = np.argsort(-deg, kind="stable")
    npc = N // NCORES
    T = (npc + P - 1) // P
    npad = T * P

    pos = np.empty(N, dtype=np.int64)
    cores_of = order[:npc * NCORES].reshape(npc, NCORES)   # [j, k]
    for k in range(NCORES):
        pos[cores_of[:, k]] = k * npad + np.arange(npc)

    deg_sorted = deg[order]
    D = [max(1, int(deg_sorted[min(t * P * NCORES, N - 1)])) for t in range(T)]

    sort_by_dst = np.argsort(dst, kind="stable")
    src_sorted = src[sort_by_dst]
    starts = np.zeros(N + 1, dtype=np.int64)
    np.cumsum(deg, out=starts[1:])

    per_core = []
    for k in range(NCORES):
        nodes_k = np.concatenate(
            [cores_of[:, k], np.full(npad - npc, order[-1], dtype=np.int64)])
        deg_k = deg[nodes_k].astype(np.float32)
        deg_k[npc:] = 1.0
        tabs = []
        for t in range(T):
            nt = nodes_k[t * P:(t + 1) * P]
            Dt = D[t]
            tab = np.zeros((P, Dt), dtype=np.int64)
            for p, n in enumerate(nt):
                s, e = starts[n], starts[n + 1]
                d = min(int(e - s), Dt)
                tab[p, :d] = src_sorted[s:s + d]
            tabs.append(tab)
        per_core.append(dict(
            nodes=nodes_k,
            deg_pt=np.ascontiguousarray(deg_k.reshape(T, P).T),
            tabs=tabs,                       # per-tile [128, D_t] src node ids
        ))
    meta = dict(N=N, T=T, npad=npad, D=D, npc=npc, pos=pos)
    return meta, per_core


def _expand(source, per_core, meta, pos_space):
    """Halo exchange: per core, gather source rows into slot-grid order.

    source: [N, F] (pos_space=False, raw node ids) or [npad*8, F]
    (pos_space=True, AllGather positions). Returns list of flat arrays.
    """
    pos = meta["pos"]
    out = []
    for pc in per_core:
        parts = []
        for tab in pc["tabs"]:
            idx = pos[tab] if pos_space else tab
            parts.append(source[idx].reshape(-1))
        out.append(np.ascontiguousarray(
            np.concatenate(parts).astype(np.float32)))
    return out


def _rep(v, rows=P):
    v = np.asarray(v, dtype=np.float32).reshape(1, -1)
    return np.ascontiguousarray(np.repeat(v, rows, axis=0))


def _fold_bn(b, g, be, rm, rv):
    s = g / np.sqrt(rv + EPS)
    return s.astype(np.float32), ((b - rm) * s + be).astype(np.float32)


# ------------------------------------------------------------- device build

def _edge_phase(nc, sb, layer, meta, F_src, keep, exp_dram,
                adst_sb, deg_sb, iota_sb, agg_sb, rec_sb,
                ws_sb=None, scal_col=None):
    """One layer's edge phase, reading halo rows affinely per tile."""
    T, D = meta["T"], meta["D"]
    off = 0
    for t in range(T):
        Dt = D[t]
        G = sb.tile([P, Dt, F_src], dt.float32, tag="G")
        nc.sync.dma_start(
            out=G[:],
            in_=exp_dram[off:off + P * Dt * F_src]
                .rearrange("(p d f) -> p d f", p=P, d=Dt))
        off += P * Dt * F_src

        if layer == 1:
            prod = sb.tile([P, Dt, F_src], dt.float32, tag="wG")
            nc.vector.tensor_tensor(
                out=prod[:], in0=G[:],
                in1=ws_sb[:, None, :].to_broadcast([P, Dt, F_src]),
                op=mybir.AluOpType.mult)
            asrc = sb.tile([P, Dt], dt.float32, tag="asrc")
            nc.vector.tensor_reduce(out=asrc[:], in_=prod[:],
                                    axis=mybir.AxisListType.X,
                                    op=mybir.AluOpType.add)
            asrc_ap = asrc[:]
        else:
            asrc_ap = G[:, :, scal_col]

        mask = sb.tile([P, Dt], dt.float32, tag="mask")
        nc.vector.tensor_scalar(out=mask[:], in0=iota_sb[:, :Dt],
                                scalar1=deg_sb[:, t:t + 1], scalar2=None,
                                op0=mybir.AluOpType.is_lt)
        maskneg = sb.tile([P, Dt], dt.float32, tag="maskneg")
        nc.vector.tensor_scalar(out=maskneg[:], in0=mask[:],
                                scalar1=1.0, scalar2=BIG,
                                op0=mybir.AluOpType.subtract,
                                op1=mybir.AluOpType.mult)
        z = sb.tile([P, Dt], dt.float32, tag="z")
        nc.vector.scalar_tensor_tensor(
            out=z[:], in0=asrc_ap, scalar=adst_sb[:, t:t + 1], in1=maskneg[:],
            op0=mybir.AluOpType.add, op1=mybir.AluOpType.add)
        lr = sb.tile([P, Dt], dt.float32, tag="lr")
        nc.vector.scalar_tensor_tensor(
            out=lr[:], in0=z[:], scalar=NEG_SLOPE, in1=z[:],
            op0=mybir.AluOpType.mult, op1=mybir.AluOpType.max)
        e = sb.tile([P, Dt], dt.float32, tag="e")
        denom = sb.tile([P, 1], dt.float32, tag="denom")
        nc.scalar.activation(out=e[:], in_=lr[:],
                             func=mybir.ActivationFunctionType.Exp,
                             accum_out=denom[:])
        nc.vector.reciprocal(out=rec_sb[:, t:t + 1], in_=denom[:])

        wG = sb.tile([P, Dt, F_src], dt.float32, tag="wG")
        nc.vector.tensor_tensor(
            out=wG[:], in0=G[:],
            in1=e[:, :, None].to_broadcast([P, Dt, F_src]),
            op=mybir.AluOpType.mult)
        h = Dt
        while h > 1:
            a = h // 2
            nc.vector.tensor_tensor(
                out=wG[:, :a, :], in0=wG[:, :a, :], in1=wG[:, a:2 * a, :],
                op=mybir.AluOpType.add)
            if h % 2:
                nc.vector.tensor_tensor(
                    out=wG[:, :1, :], in0=wG[:, :1, :], in1=wG[:, h - 1:h, :],
                    op=mybir.AluOpType.add)
            h = a
        nc.vector.tensor_copy(out=agg_sb[:, t, :], in_=wG[:, 0, :keep])


def _common_prelude(nc, pe_, T, Dmax, consts):
    from concourse.masks import make_identity
    ident = pe_.tile([P, P], dt.float32, tag="c_id")
    make_identity(nc, ident[:])
    sbufs = {}
    for name, (drt, shape) in consts.items():
        tl = pe_.tile(shape, dt.float32, tag="c_" + name)
        nc.sync.dma_start(out=tl[:], in_=drt[:])
        sbufs[name] = tl
    return ident, sbufs


def build_layer1(meta):
    """x_exp -> edge phase (dot asrc) -> dense -> x2e [npad, XW] output."""
    T, npad, D = meta["T"], meta["npad"], meta["D"]
    Dmax = max(D)
    nE = sum(D) * P

    nc = bacc.Bacc("TRN2", target_bir_lowering=False, debug=False,
                   enable_asserts=True, num_devices=NCORES)
    x_exp = nc.dram_tensor("x_exp", [nE * F_IN], dt.float32, kind="ExternalInput")
    x_own = nc.dram_tensor("x_own", [npad, F_IN], dt.float32, kind="ExternalInput")
    deg_pt = nc.dram_tensor("deg_pt", [P, T], dt.float32, kind="ExternalInput")
    iota = nc.dram_tensor("iota", [P, Dmax], dt.float32, kind="ExternalInput")
    w1 = nc.dram_tensor("w1", [F_IN, H1], dt.float32, kind="ExternalInput")
    ws1 = nc.dram_tensor("ws1", [P, F_IN], dt.float32, kind="ExternalInput")
    wd1 = nc.dram_tensor("wd1", [P, F_IN], dt.float32, kind="ExternalInput")
    ws2 = nc.dram_tensor("ws2", [P, H1], dt.float32, kind="ExternalInput")
    wd2 = nc.dram_tensor("wd2", [P, H1], dt.float32, kind="ExternalInput")
    sc1 = nc.dram_tensor("sc1", [P, H1], dt.float32, kind="ExternalInput")
    sh1 = nc.dram_tensor("sh1", [P, H1], dt.float32, kind="ExternalInput")
    x2e = nc.dram_tensor("x2e", [npad, XW], dt.float32, kind="ExternalOutput")

    with tile.TileContext(nc) as tc:
        with tc.tile_pool(name="sbuf", bufs=3) as sb, \
             tc.tile_pool(name="gth", bufs=2) as gth, \
             tc.tile_pool(name="persist", bufs=1) as pe_, \
             tc.tile_pool(name="psum", bufs=2, space="PSUM") as ps:
            ident, cs = _common_prelude(nc, pe_, T, Dmax, dict(
                iota=(iota, [P, Dmax]), deg=(deg_pt, [P, T]),
                ws1=(ws1, [P, F_IN]), wd1=(wd1, [P, F_IN]),
                ws2=(ws2, [P, H1]), wd2=(wd2, [P, H1]),
                sc1=(sc1, [P, H1]), sh1=(sh1, [P, H1]),
                w1=(w1, [F_IN, H1])))

            agg_sb = pe_.tile([P, T, H1], dt.float32, tag="agg")
            rec_sb = pe_.tile([P, T], dt.float32, tag="rec")
            adst_sb = pe_.tile([P, T], dt.float32, tag="adst")

            for t in range(T):
                xo = sb.tile([P, F_IN], dt.float32, tag="xo")
                nc.sync.dma_start(out=xo[:], in_=x_own[t * P:(t + 1) * P, :])
                scr = sb.tile([P, F_IN], dt.float32, tag="scr")
                nc.vector.tensor_tensor(out=scr[:], in0=xo[:],
                                        in1=cs["wd1"][:],
                                        op=mybir.AluOpType.mult)
                nc.vector.tensor_reduce(out=adst_sb[:, t:t + 1], in_=scr[:],
                                        axis=mybir.AxisListType.X,
                                        op=mybir.AluOpType.add)

            _edge_phase(nc, gth, 1, meta, F_IN, H1, x_exp,
                        adst_sb, cs["deg"], cs["iota"], agg_sb, rec_sb,
                        ws_sb=cs["ws1"])

            for t in range(T):
                xt = sb.tile([P, F_IN], dt.float32, tag="xt")
                nc.vector.tensor_scalar(out=xt[:], in0=agg_sb[:, t, :],
                                        scalar1=rec_sb[:, t:t + 1],
                                        scalar2=None, op0=mybir.AluOpType.mult)
                xtT_ps = ps.tile([P, P], dt.float32, tag="tps")
                nc.tensor.transpose(out=xtT_ps[:], in_=xt[:], identity=ident[:])
                xtT = sb.tile([P, P], dt.float32, tag="xtT")
                nc.vector.tensor_copy(out=xtT[:], in_=xtT_ps[:])
                mm = ps.tile([P, H1], dt.float32, tag="mm")
                nc.tensor.matmul(out=mm[:], lhsT=xtT[:], rhs=cs["w1"][:],
                                 start=True, stop=True)
                x2 = sb.tile([P, H1], dt.float32, tag="x2")
                nc.vector.tensor_tensor(out=x2[:], in0=mm[:], in1=cs["sc1"][:],
                                        op=mybir.AluOpType.mult)
                nc.vector.tensor_tensor(out=x2[:], in0=x2[:], in1=cs["sh1"][:],
                                        op=mybir.AluOpType.add)
                nc.scalar.activation(out=x2[:], in_=x2[:],
                                     func=mybir.ActivationFunctionType.Tanh)
                scr = sb.tile([P, H1], dt.float32, tag="scr")
                sc_col = sb.tile([P, 4], dt.float32, tag="sc_col")
                nc.vector.memset(sc_col[:], 0.0)
                nc.vector.tensor_tensor(out=scr[:], in0=x2[:],
                                        in1=cs["ws2"][:],
                                        op=mybir.AluOpType.mult)
                nc.vector.tensor_reduce(out=sc_col[:, 0:1], in_=scr[:],
                                        axis=mybir.AxisListType.X,
                                        op=mybir.AluOpType.add)
                nc.vector.tensor_tensor(out=scr[:], in0=x2[:],
                                        in1=cs["wd2"][:],
                                        op=mybir.AluOpType.mult)
                nc.vector.tensor_reduce(out=sc_col[:, 1:2], in_=scr[:],
                                        axis=mybir.AxisListType.X,
                                        op=mybir.AluOpType.add)
                nc.sync.dma_start(out=x2e[t * P:(t + 1) * P, 0:H1], in_=x2[:])
                nc.sync.dma_start(out=x2e[t * P:(t + 1) * P, H1:XW],
                                  in_=sc_col[:])
    nc.compile()
    return nc


def build_layer2(meta):
    """x2exp -> edge phase (packed asrc) -> dense -> h3e [npad, CP] output."""
    T, npad, D = meta["T"], meta["npad"], meta["D"]
    Dmax = max(D)
    nE = sum(D) * P

    nc = bacc.Bacc("TRN2", target_bir_lowering=False, debug=False,
                   enable_asserts=True, num_devices=NCORES)
    x2exp = nc.dram_tensor("x2exp", [nE * XW], dt.float32, kind="ExternalInput")
    adst = nc.dram_tensor("adst", [P, T], dt.float32, kind="ExternalInput")
    deg_pt = nc.dram_tensor("deg_pt", [P, T], dt.float32, kind="ExternalInput")
    iota = nc.dram_tensor("iota", [P, Dmax], dt.float32, kind="ExternalInput")
    w2 = nc.dram_tensor("w2", [H1, H2], dt.float32, kind="ExternalInput")
    w3e = nc.dram_tensor("w3e", [H2, CP], dt.float32, kind="ExternalInput")
    sc2 = nc.dram_tensor("sc2", [P, H2], dt.float32, kind="ExternalInput")
    sh2 = nc.dram_tensor("sh2", [P, H2], dt.float32, kind="ExternalInput")
    h3e = nc.dram_tensor("h3e", [npad, CP], dt.float32, kind="ExternalOutput")

    with tile.TileContext(nc) as tc:
        with tc.tile_pool(name="sbuf", bufs=3) as sb, \
             tc.tile_pool(name="gth", bufs=2) as gth, \
             tc.tile_pool(name="persist", bufs=1) as pe_, \
             tc.tile_pool(name="psum", bufs=2, space="PSUM") as ps:
            ident, cs = _common_prelude(nc, pe_, T, Dmax, dict(
                iota=(iota, [P, Dmax]), deg=(deg_pt, [P, T]),
                adst=(adst, [P, T]),
                sc2=(sc2, [P, H2]), sh2=(sh2, [P, H2]),
                w2=(w2, [H1, H2]),
                w3a=(w3e[0:P, :], [P, CP]), w3b=(w3e[P:H2, :], [P, CP])))

            agg_sb = pe_.tile([P, T, H1], dt.float32, tag="agg")
            rec_sb = pe_.tile([P, T], dt.float32, tag="rec")

            _edge_phase(nc, gth, 2, meta, XW, H1, x2exp,
                        cs["adst"], cs["deg"], cs["iota"], agg_sb, rec_sb,
                        scal_col=H1)

            for t in range(T):
                xt = sb.tile([P, H1], dt.float32, tag="xt")
                nc.vector.tensor_scalar(out=xt[:], in0=agg_sb[:, t, :],
                                        scalar1=rec_sb[:, t:t + 1],
                                        scalar2=None, op0=mybir.AluOpType.mult)
                xtT_ps = ps.tile([P, P], dt.float32, tag="tps")
                nc.tensor.transpose(out=xtT_ps[:], in_=xt[:], identity=ident[:])
                xtT = sb.tile([P, P], dt.float32, tag="xtT")
                nc.vector.tensor_copy(out=xtT[:], in_=xtT_ps[:])
                mm2 = ps.tile([P, H2], dt.float32, tag="mm")
                nc.tensor.matmul(out=mm2[:], lhsT=xtT[:], rhs=cs["w2"][:],
                                 start=True, stop=True)
                x3 = sb.tile([P, H2], dt.float32, tag="x3")
                nc.vector.tensor_tensor(out=x3[:], in0=mm2[:], in1=cs["sc2"][:],
                                        op=mybir.AluOpType.mult)
                nc.vector.tensor_tensor(out=x3[:], in0=x3[:], in1=cs["sh2"][:],
                                        op=mybir.AluOpType.add)
                nc.scalar.activation(out=x3[:], in_=x3[:],
                                     func=mybir.ActivationFunctionType.Tanh)
                xT0_ps = ps.tile([P, P], dt.float32, tag="tps")
                nc.tensor.transpose(out=xT0_ps[:], in_=x3[:, 0:P],
                                    identity=ident[:])
                xT0 = sb.tile([P, P], dt.float32, tag="xtT")
                nc.vector.tensor_copy(out=xT0[:], in_=xT0_ps[:])
                xT1_ps = ps.tile([P, P], dt.float32, tag="tps")
                nc.tensor.transpose(out=xT1_ps[:], in_=x3[:, P:H2],
                                    identity=ident[:])
                xT1 = sb.tile([P, P], dt.float32, tag="xtT1")
                nc.vector.tensor_copy(out=xT1[:], in_=xT1_ps[:])
                h3ps = ps.tile([P, CP], dt.float32, tag="mm")
                nc.tensor.matmul(out=h3ps[:], lhsT=xT0[:], rhs=cs["w3a"][:],
                                 start=True, stop=False)
                nc.tensor.matmul(out=h3ps[:], lhsT=xT1[:], rhs=cs["w3b"][:],
                                 start=False, stop=True)
                h3 = sb.tile([P, CP], dt.float32, tag="h3")
                nc.vector.tensor_copy(out=h3[:], in_=h3ps[:])
                nc.sync.dma_start(out=h3e[t * P:(t + 1) * P, :], in_=h3[:])
    nc.compile()
    return nc


def build_layer3(meta):
    """h3exp -> edge phase (packed asrc) -> +b3 -> o [npad, C] output."""
    T, npad, D = meta["T"], meta["npad"], meta["D"]
    Dmax = max(D)
    nE = sum(D) * P

    nc = bacc.Bacc("TRN2", target_bir_lowering=False, debug=False,
                   enable_asserts=True, num_devices=NCORES)
    h3exp = nc.dram_tensor("h3exp", [nE * CP], dt.float32, kind="ExternalInput")
    adst = nc.dram_tensor("adst", [P, T], dt.float32, kind="ExternalInput")
    deg_pt = nc.dram_tensor("deg_pt", [P, T], dt.float32, kind="ExternalInput")
    iota = nc.dram_tensor("iota", [P, Dmax], dt.float32, kind="ExternalInput")
    b3r = nc.dram_tensor("b3r", [P, C], dt.float32, kind="ExternalInput")
    o = nc.dram_tensor("o", [npad, C], dt.float32, kind="ExternalOutput")

    with tile.TileContext(nc) as tc:
        with tc.tile_pool(name="sbuf", bufs=3) as sb, \
             tc.tile_pool(name="gth", bufs=2) as gth, \
             tc.tile_pool(name="persist", bufs=1) as pe_:
            from concourse.masks import make_identity
            cs = {}
            for name, (drt, shape) in dict(
                    iota=(iota, [P, Dmax]), deg=(deg_pt, [P, T]),
                    adst=(adst, [P, T]), b3=(b3r, [P, C])).items():
                tl = pe_.tile(shape, dt.float32, tag="c_" + name)
                nc.sync.dma_start(out=tl[:], in_=drt[:])
                cs[name] = tl

            agg_sb = pe_.tile([P, T, C], dt.float32, tag="agg")
            rec_sb = pe_.tile([P, T], dt.float32, tag="rec")

            _edge_phase(nc, gth, 3, meta, CP, C, h3exp,
                        cs["adst"], cs["deg"], cs["iota"], agg_sb, rec_sb,
                        scal_col=C)

            for t in range(T):
                ot = sb.tile([P, C], dt.float32, tag="ot")
                nc.vector.tensor_scalar(out=ot[:], in0=agg_sb[:, t, :],
                                        scalar1=rec_sb[:, t:t + 1],
                                        scalar2=None, op0=mybir.AluOpType.mult)
                nc.vector.tensor_tensor(out=ot[:], in0=ot[:], in1=cs["b3"][:],
                                        op=mybir.AluOpType.add)
                nc.sync.dma_start(out=o[t * P:(t + 1) * P, :], in_=ot[:])
    nc.compile()
    return nc


# ------------------------------------------------------------------ kernel

_BUILD_CACHE = {}


def _get_programs(meta):
    key = (meta["N"], tuple(meta["D"]))
    if key not in _BUILD_CACHE:
        _BUILD_CACHE[key] = (build_layer1(meta), build_layer2(meta),
                             build_layer3(meta))
    return _BUILD_CACHE[key]


def run_all(inputs, meta, per_core, x, collect_times=False):
    T, npad, npc = meta["T"], meta["npad"], meta["npc"]
    N = meta["N"]
    Dmax = max(meta["D"])
    g = lambda n: np.asarray(inputs[n], np.float32)
    w1, w2, w3 = g("w1"), g("w2"), g("w3")
    sc1, sh1 = _fold_bn(g("b1"), g("g1"), g("be1"), g("rm1"), g("rv1"))
    sc2, sh2 = _fold_bn(g("b2"), g("g2"), g("be2"), g("rm2"), g("rv2"))
    w3e = np.zeros((H2, CP), np.float32)
    w3e[:, :C] = w3
    w3e[:, C] = w3 @ g("as3")
    w3e[:, C + 1] = w3 @ g("ad3")
    iota_row = _rep(np.arange(Dmax, dtype=np.float32))

    ncA, ncB, ncC = _get_programs(meta)
    import time
    times = []

    # ---- layer 1 ----
    x_exp = _expand(x, per_core, meta, pos_space=False)
    maps = []
    for k in range(NCORES):
        pc = per_core[k]
        maps.append(dict(
            x_exp=x_exp[k], x_own=np.ascontiguousarray(x[pc["nodes"]]),
            deg_pt=pc["deg_pt"], iota=iota_row,
            w1=w1, ws1=_rep(w1 @ g("as1")), wd1=_rep(w1 @ g("ad1")),
            ws2=_rep(w2 @ g("as2")), wd2=_rep(w2 @ g("ad2")),
            sc1=_rep(sc1), sh1=_rep(sh1)))
    t0 = time.perf_counter()
    brA = bass_utils.run_bass_kernel_spmd(ncA, maps, list(range(NCORES)))
    times.append(time.perf_counter() - t0)
    x2e_full = np.concatenate([brA.results[k]["x2e"] for k in range(NCORES)])

    # ---- layer 2 ----
    x2exp = _expand(x2e_full, per_core, meta, pos_space=True)
    maps = []
    for k in range(NCORES):
        pc = per_core[k]
        adst2 = x2e_full[k * npad:(k + 1) * npad, H1 + 1].astype(np.float32)
        maps.append(dict(
            x2exp=x2exp[k],
            adst=np.ascontiguousarray(adst2.reshape(T, P).T),
            deg_pt=pc["deg_pt"], iota=iota_row,
            w2=w2, w3e=w3e, sc2=_rep(sc2), sh2=_rep(sh2)))
    t0 = time.perf_counter()
    brB = bass_utils.run_bass_kernel_spmd(ncB, maps, list(range(NCORES)))
    times.append(time.perf_counter() - t0)
    h3_full = np.concatenate([brB.results[k]["h3e"] for k in range(NCORES)])

    # ---- layer 3 ----
    h3exp = _expand(h3_full, per_core, meta, pos_space=True)
    maps = []
    for k in range(NCORES):
        pc = per_core[k]
        adst3 = h3_full[k * npad:(k + 1) * npad, C + 1].astype(np.float32)
        maps.append(dict(
            h3exp=h3exp[k],
            adst=np.ascontiguousarray(adst3.reshape(T, P).T),
            deg_pt=pc["deg_pt"], iota=iota_row, b3r=_rep(g("b3"))))
    t0 = time.perf_counter()
    brC = bass_utils.run_bass_kernel_spmd(ncC, maps, list(range(NCORES)))
    times.append(time.perf_counter() - t0)

    out = np.empty((N, C), dtype=np.float32)
    for k in range(NCORES):
        out[per_core[k]["nodes"][:npc]] = brC.results[k]["o"][:npc]
    if collect_times:
        return out, times
    return out


def kernel(**inputs):
    x = np.ascontiguousarray(np.asarray(inputs["x"], dtype=np.float32))
    meta, per_core = _prep(x, inputs["edge_index"])
    return run_all(inputs, meta, per_core, x)
